# revision 8
# baseline (speedup 1.0000x reference)
"""Trainium2 Bass kernel for nn_DeepDownstreamFork (dense MLP chain + skips + layernorm).

Per batch b of 8 (one NeuronCore each):
    big = relu(x @ (W_large * S_large).T)           # [T, L]   T=4096, H=1024, L=4096
    big = relu(big @ W_c1.T)                        # [T, L]
    big = big @ W_c2.T                              # [T, H]
    s   = x @ (W_s1 * S_s1 + W_s2 * S_s2).T         # [T, H]
    out = layernorm(big + s) * gamma + beta         # [T, H]

Data-parallel over batch (8 cores, no comm).  Host passes pre-TRANSPOSED
tensors (xT, W*T) so every SBUF operand loads with the contraction dim on
partitions via plain strided DMA -- no xbar transposes at all.  Tile 0
dequants/casts each weight stripe just-in-time (f32 load -> vector scale ->
bf16/fp8), uses it, and stores it back to DRAM scratch; tiles 1..7 reload the
scratch stripes.  DMA is spread over three queues (sync/scalar HWDGE +
gpsimd SWDGE casts).

Phase B (the 137 GF matmul) runs the first NF8 of its 32 contraction
k-blocks in fp8e4 DoubleRow (2 k-blocks per PE pass).  Activations are
scaled by 2^-5 and Wc1 by 2^5 when quantizing, so fp8 partial products have
scale 1 and accumulate into the same PSUM group as the bf16 k-blocks.
"""

import os
import sys

import numpy as np

if os.path.isdir("/opt/trn_rl_repo") and "/opt/trn_rl_repo" not in sys.path:
    sys.path.insert(0, "/opt/trn_rl_repo")

P = 128
H = 1024          # hidden
L = 4096          # large dim
T = 4096          # tokens per core
NCORES = 8
TT = 512          # token tile
NTT = T // TT     # 8
HO = H // P       # 8
LO = L // P       # 32
NF8 = 24          # how many of B's 32 k-blocks run in fp8 DoubleRow (even)
NBF = LO - NF8    # bf16 k-blocks in B
SY = 1.0 / 32.0   # y1 fp8 quant scale (2^-5)
SW = 32.0         # Wc1 fp8 quant scale (2^5); SY*SW == 1

_CACHED_NC = None


def _build_nc():
    import concourse.mybir as mybir
    import concourse.tile as tile
    from concourse import bacc

    F32 = mybir.dt.float32
    BF16 = mybir.dt.bfloat16
    FP8 = mybir.dt.float8e4
    Relu = mybir.ActivationFunctionType.Relu
    Sqrt = mybir.ActivationFunctionType.Sqrt
    Copy = mybir.ActivationFunctionType.Copy
    MUL = mybir.AluOpType.mult
    SUB = mybir.AluOpType.subtract
    ADD = mybir.AluOpType.add
    DR = mybir.MatmulPerfMode.DoubleRow

    nc = bacc.Bacc(
        None, target_bir_lowering=False, debug=False,
        dynamic_dma_scratch_size=32768,
    )

    xT = nc.dram_tensor("xT", [H, T], F32, kind="ExternalInput")
    WLT = nc.dram_tensor("WLT", [H, L], F32, kind="ExternalInput")
    Wc1T = nc.dram_tensor("Wc1T", [L, L], F32, kind="ExternalInput")
    Wc2T = nc.dram_tensor("Wc2T", [L, H], F32, kind="ExternalInput")
    Ws1T = nc.dram_tensor("Ws1T", [H, H], F32, kind="ExternalInput")
    Ws2T = nc.dram_tensor("Ws2T", [H, H], F32, kind="ExternalInput")
    gamma = nc.dram_tensor("gamma", [H], F32, kind="ExternalInput")
    beta = nc.dram_tensor("beta", [H], F32, kind="ExternalInput")
    s_largeT = nc.dram_tensor("s_largeT", [HO, LO], F32, kind="ExternalInput")
    s_s1T = nc.dram_tensor("s_s1T", [HO, HO], F32, kind="ExternalInput")
    s_s2T = nc.dram_tensor("s_s2T", [HO, HO], F32, kind="ExternalInput")
    out = nc.dram_tensor("out", [T, H], F32, kind="ExternalOutput")

    def tslab(dram_t, rows_off, nrows_k, col_off, width):
        # [128, nrows_k, width] view: partition p <- row k*128+p (+rows_off)
        return dram_t[:].rearrange("(k p) c -> p k c", p=P)[
            :, rows_off // P:rows_off // P + nrows_k, col_off:col_off + width
        ]

    with tile.TileContext(nc) as tc:
        with (
            tc.tile_pool(name="dram", bufs=1, space="DRAM") as dram,
            tc.tile_pool(name="consts", bufs=1) as consts,
            tc.tile_pool(name="wpool", bufs=2) as wpool,
            tc.tile_pool(name="apool", bufs=1) as apool,
            tc.tile_pool(name="lnp", bufs=4) as lnp,
            tc.tile_pool(name="psum", bufs=6, space="PSUM") as psum,
        ):
            # ---- DRAM scratch (dequanted / cast weight stripes) ----
            WTl = dram.tile([8, P, HO, 512], BF16)
            WTc1b = dram.tile([16, P, NBF, 256], BF16)
            WTc18 = dram.tile([16, P, NF8, 256], FP8)
            WTc2 = dram.tile([4, P, LO, 256], BF16)
            WTs = dram.tile([4, P, HO, 256], BF16)
            xbf = dram.tile([H, T], BF16)

            # one-time contiguous x cast slabs (cheap SWDGE descriptors);
            # tiles 1..7 then load x^T tiles from xbf via sync HWDGE
            for i in range(HO):
                nc.gpsimd.dma_start(xbf[i * P:(i + 1) * P, :], xT[i * P:(i + 1) * P, :])

            # ---- constants ----
            scA = consts.tile([P, HO, LO], F32)      # s_large^T  [k_h, l_blk]
            nc.sync.dma_start(
                scA[:], s_largeT[:].rearrange("a b -> (a b)")[None, :]
                .to_broadcast([P, HO * LO]).rearrange("p (a b) -> p a b", b=LO)
            )
            sc1 = consts.tile([P, HO, HO], F32)
            nc.sync.dma_start(
                sc1[:], s_s1T[:].rearrange("a b -> (a b)")[None, :]
                .to_broadcast([P, HO * HO]).rearrange("p (a b) -> p a b", b=HO)
            )
            sc2 = consts.tile([P, HO, HO], F32)
            nc.sync.dma_start(
                sc2[:], s_s2T[:].rearrange("a b -> (a b)")[None, :]
                .to_broadcast([P, HO * HO]).rearrange("p (a b) -> p a b", b=HO)
            )
            gamma_b = consts.tile([P, H], F32)
            nc.sync.dma_start(gamma_b[:], gamma[:][None, :].to_broadcast([P, H]))
            beta_b = consts.tile([P, H], F32)
            nc.sync.dma_start(beta_b[:], beta[:][None, :].to_broadcast([P, H]))
            eps_t = consts.tile([P, 1], F32)
            nc.vector.memset(eps_t[:], 1e-5)

            for tt in range(NTT):
                t0 = tt * TT
                first = tt == 0
                # x^T tile [h_in-part, h_blk, t] bf16, SWDGE cast-load
                xt = apool.tile([P, HO, TT], BF16, tag="xt", bufs=2, name="xt")
                if tt == 0:
                    nc.gpsimd.dma_start(xt[:], tslab(xT, 0, HO, t0, TT))
                else:
                    nc.sync.dma_start(xt[:], tslab(xbf, 0, HO, t0, TT))

                # ---- Phase A: y1T[l, t] = relu(sum_h WLT[h, l] * xT[h, t]) ----
                y1b = apool.tile([P, NBF, TT], BF16, tag="y1b", bufs=1, name="y1b")
                y18 = apool.tile([P, NF8, TT], FP8, tag="y18", bufs=1, name="y18")
                with nc.named_scope(f"A{tt}"):
                    for j in range(8):          # l-stripes of 512
                        wl = wpool.tile([P, HO, 512], BF16, tag="wl", bufs=3, name="wl")
                        if first:
                            for hh in range(2):     # half-stripes of 256
                                t0a = wpool.tile([P, HO, 256], F32, tag="t0",
                                                 bufs=2, name="t0a")
                                nc.sync.dma_start(
                                    t0a[:], tslab(WLT, 0, HO, j * 512 + hh * 256, 256))
                                nc.vector.tensor_tensor(
                                    wl[:, :, hh * 256:(hh + 1) * 256]
                                    .rearrange("p k (b c) -> p k b c", c=P),
                                    t0a[:].rearrange("p k (b c) -> p k b c", c=P),
                                    scA[:, :, j * 4 + hh * 2:j * 4 + hh * 2 + 2]
                                    [:, :, :, None].to_broadcast([P, HO, 2, P]),
                                    MUL,
                                )
                            nc.sync.dma_start(WTl[j], wl[:])
                        else:
                            nc.sync.dma_start(wl[:], WTl[j])
                        for m in range(4):      # l-chunks of 128
                            jg = j * 4 + m      # B-contraction k-block id
                            ps = psum.tile([P, TT], F32, tag="ps", name="psA")
                            for k in range(HO):
                                nc.tensor.matmul(
                                    ps[:], wl[:, k, m * P:(m + 1) * P], xt[:, k, :],
                                    start=(k == 0), stop=(k == HO - 1),
                                )
                            if jg < NF8:
                                nc.scalar.activation(y18[:, jg, :], ps[:], Relu,
                                                     scale=SY)
                            else:
                                nc.scalar.activation(y1b[:, jg - NF8, :], ps[:], Relu)

                # ---- Phase B: y2T[l2, t] = relu(sum_l1 Wc1T[l1, l2] * y1T[l1, t]) ----
                y2T = apool.tile([P, LO, TT], BF16, tag="y2", bufs=1, name="y2T")
                with nc.named_scope(f"B{tt}"):
                    for j2 in range(16):        # l2-stripes of 256
                        wcb = wpool.tile([P, NBF, 256], BF16, tag="wcb", name="wcb")
                        wc8 = wpool.tile([P, NF8, 256], FP8, tag="wc8", bufs=1, name="wc8")
                        if first:
                            nc.gpsimd.dma_start(
                                wcb[:], tslab(Wc1T, NF8 * P, NBF, j2 * 256, 256))
                            nc.gpsimd.dma_start(WTc1b[j2], wcb[:])
                            for hh in range(NF8 // 8):
                                t0b = wpool.tile([P, 8, 256], F32, tag="t0",
                                                 bufs=2, name="t0b")
                                nc.sync.dma_start(
                                    t0b[:], tslab(Wc1T, hh * 8 * P, 8, j2 * 256, 256))
                                nc.scalar.activation(
                                    wc8[:, hh * 8:(hh + 1) * 8, :], t0b[:], Copy,
                                    scale=SW)
                            nc.gpsimd.dma_start(WTc18[j2], wc8[:])
                        else:
                            nc.gpsimd.dma_start(wcb[:], WTc1b[j2])
                            nc.gpsimd.dma_start(wc8[:], WTc18[j2])
                        for m in range(2):      # l2-chunks of 128
                            ps = psum.tile([P, TT], F32, tag="ps", name="psB")
                            for kp in range(0, NF8, 2):
                                nc.tensor.matmul(
                                    ps[:], wc8[:, kp:kp + 2, m * P:(m + 1) * P],
                                    y18[:, kp:kp + 2, :],
                                    start=(kp == 0), stop=False, perf_mode=DR,
                                )
                            for k in range(NBF):
                                nc.tensor.matmul(
                                    ps[:], wcb[:, k, m * P:(m + 1) * P], y1b[:, k, :],
                                    start=False, stop=(k == NBF - 1),
                                )
                            nc.scalar.activation(y2T[:, j2 * 2 + m, :], ps[:], Relu)

                # ---- Phase C: pre[t, h] = sum_l2 y2[l2, t]*Wc2T[l2, h] + skip ----
                preln = apool.tile([P, TT // P, H], F32, tag="ln", bufs=1, name="preln")
                with nc.named_scope(f"C{tt}"):
                    for jh in range(4):         # h-blocks of 256
                        wc2s = wpool.tile([P, LO, 256], BF16, tag="wc2s", name="wc2s")
                        ws = wpool.tile([P, HO, 256], BF16, tag="ws", bufs=1, name="ws")
                        if first:
                            nc.gpsimd.dma_start(
                                wc2s[:], tslab(Wc2T, 0, LO, jh * 256, 256))
                            nc.gpsimd.dma_start(WTc2[jh], wc2s[:])
                            w1 = wpool.tile([P, HO, 256], F32, tag="t0", bufs=2,
                                            name="w1")
                            nc.sync.dma_start(
                                w1[:], tslab(Ws1T, 0, HO, jh * 256, 256))
                            w2 = wpool.tile([P, HO, 256], F32, tag="t0", bufs=2,
                                            name="w2")
                            nc.sync.dma_start(
                                w2[:], tslab(Ws2T, 0, HO, jh * 256, 256))
                            for w, sc in ((w1, sc1), (w2, sc2)):
                                nc.vector.tensor_tensor(
                                    w[:].rearrange("p k (b c) -> p k b c", c=P),
                                    w[:].rearrange("p k (b c) -> p k b c", c=P),
                                    sc[:, :, jh * 2:jh * 2 + 2]
                                    [:, :, :, None].to_broadcast([P, HO, 2, P]),
                                    MUL,
                                )
                            nc.vector.tensor_tensor(ws[:], w1[:], w2[:], ADD)
                            nc.sync.dma_start(WTs[jh], ws[:])
                        else:
                            nc.sync.dma_start(wc2s[:], WTc2[jh])
                            nc.sync.dma_start(ws[:], WTs[jh])
                        for tn in range(TT // P):   # token chunks of 128
                            ps = psum.tile([P, 256], F32, tag="ps", name="psC")
                            for k in range(LO):
                                nc.tensor.matmul(
                                    ps[:], y2T[:, k, tn * P:(tn + 1) * P],
                                    wc2s[:, k, :],
                                    start=(k == 0), stop=False,
                                )
                            for k in range(HO):
                                nc.tensor.matmul(
                                    ps[:], xt[:, k, tn * P:(tn + 1) * P], ws[:, k, :],
                                    start=False, stop=(k == HO - 1),
                                )
                            nc.vector.tensor_copy(
                                preln[:, tn, jh * 256:(jh + 1) * 256], ps[:]
                            )

                # ---- Phase D: layernorm over h, * gamma + beta ----
                with nc.named_scope(f"D{tt}"):
                    for tn in range(TT // P):
                        pslice = preln[:, tn, :]
                        st = lnp.tile([P, 2, 6], F32, name="st")
                        for g in range(2):
                            nc.vector.bn_stats(st[:, g, :], pslice[:, g * 512:(g + 1) * 512])
                        mv = lnp.tile([P, 2], F32, name="mv")
                        nc.vector.bn_aggr(mv[:], st[:])
                        std = lnp.tile([P, 1], F32, name="std")
                        nc.scalar.activation(std[:], mv[:, 1:2], Sqrt, bias=eps_t[:])
                        rstd = lnp.tile([P, 1], F32, name="rstd")
                        nc.vector.reciprocal(rstd[:], std[:])
                        nc.vector.tensor_scalar(
                            pslice, pslice, scalar1=mv[:, 0:1], scalar2=rstd[:],
                            op0=SUB, op1=MUL,
                        )
                        nc.vector.tensor_tensor(pslice, pslice, gamma_b[:], MUL)
                        nc.vector.tensor_tensor(pslice, pslice, beta_b[:], ADD)
                        nc.sync.dma_start(out[t0 + tn * P:t0 + (tn + 1) * P, :], pslice)

    nc.finalize()
    return nc


def _get_nc():
    global _CACHED_NC
    if _CACHED_NC is None:
        _CACHED_NC = _build_nc()
    return _CACHED_NC


def prepare_in_maps(inputs):
    f32 = np.float32
    x = np.asarray(inputs["x"], dtype=f32)                 # [8, T, H]
    xT_all = np.ascontiguousarray(x.transpose(0, 2, 1))    # [8, H, T]
    shared = {
        "WLT": np.ascontiguousarray(np.asarray(inputs["W_large"], f32).T),
        "Wc1T": np.ascontiguousarray(np.asarray(inputs["W_c1"], f32).T),
        "Wc2T": np.ascontiguousarray(np.asarray(inputs["W_c2"], f32).T),
        "Ws1T": np.ascontiguousarray(np.asarray(inputs["W_s1"], f32).T),
        "Ws2T": np.ascontiguousarray(np.asarray(inputs["W_s2"], f32).T),
        "gamma": np.ascontiguousarray(np.asarray(inputs["gamma"], f32)),
        "beta": np.ascontiguousarray(np.asarray(inputs["beta"], f32)),
        "s_largeT": np.ascontiguousarray(np.asarray(inputs["s_large"], f32).T),
        "s_s1T": np.ascontiguousarray(np.asarray(inputs["s_s1"], f32).T),
        "s_s2T": np.ascontiguousarray(np.asarray(inputs["s_s2"], f32).T),
    }
    return [{"xT": np.ascontiguousarray(xT_all[c]), **shared}
            for c in range(NCORES)]


def kernel(**inputs) -> np.ndarray:
    from concourse.bass_utils import run_bass_kernel_spmd

    nc = _get_nc()
    in_maps = prepare_in_maps(inputs)
    res = run_bass_kernel_spmd(nc, in_maps, core_ids=list(range(NCORES)))
    return np.stack([res.results[c]["out"] for c in range(NCORES)], axis=0)


# revision 9
# speedup vs baseline: 1.1814x; 1.1814x over previous
"""Trainium2 Bass kernel for nn_DeepDownstreamFork (dense MLP chain + skips + layernorm).

Per batch b of 8 (one NeuronCore each):
    big = relu(x @ (W_large * S_large).T)           # [T, L]   T=4096, H=1024, L=4096
    big = relu(big @ W_c1.T)                        # [T, L]
    big = big @ W_c2.T                              # [T, H]
    s   = x @ (W_s1 * S_s1 + W_s2 * S_s2).T         # [T, H]
    out = layernorm(big + s) * gamma + beta         # [T, H]

Data-parallel over batch (8 cores, no comm).  Host passes pre-TRANSPOSED
tensors (xT, W*T) so every SBUF operand loads with the contraction dim on
partitions via plain strided DMA -- no xbar transposes at all.  Tile 0
dequants/casts each weight stripe just-in-time (f32 load -> vector scale ->
bf16/fp8), uses it, and stores it back to DRAM scratch; tiles 1..7 reload the
scratch stripes.  DMA is spread over three queues (sync/scalar HWDGE +
gpsimd SWDGE casts).

Phase B (the 137 GF matmul) runs the first NF8 of its 32 contraction
k-blocks in fp8e4 DoubleRow (2 k-blocks per PE pass).  Activations are
scaled by 2^-5 and Wc1 by 2^5 when quantizing, so fp8 partial products have
scale 1 and accumulate into the same PSUM group as the bf16 k-blocks.
"""

import os
import sys

import numpy as np

if os.path.isdir("/opt/trn_rl_repo") and "/opt/trn_rl_repo" not in sys.path:
    sys.path.insert(0, "/opt/trn_rl_repo")

P = 128
H = 1024          # hidden
L = 4096          # large dim
T = 4096          # tokens per core
NCORES = 8
TT = 512          # token tile
NTT = T // TT     # 8
HO = H // P       # 8
LO = L // P       # 32
NF8 = 16          # how many of B's 32 k-blocks run in fp8 DoubleRow (even)
NBF = LO - NF8    # bf16 k-blocks in B
SY = 1.0 / 32.0   # y1 fp8 quant scale (2^-5)
SW = 32.0         # Wc1 fp8 quant scale (2^5); SY*SW == 1

_CACHED_NC = None


def _build_nc():
    import concourse.mybir as mybir
    import concourse.tile as tile
    from concourse import bacc

    F32 = mybir.dt.float32
    BF16 = mybir.dt.bfloat16
    FP8 = mybir.dt.float8e4
    Relu = mybir.ActivationFunctionType.Relu
    Sqrt = mybir.ActivationFunctionType.Sqrt
    Copy = mybir.ActivationFunctionType.Copy
    MUL = mybir.AluOpType.mult
    SUB = mybir.AluOpType.subtract
    ADD = mybir.AluOpType.add
    DR = mybir.MatmulPerfMode.DoubleRow

    nc = bacc.Bacc(
        None, target_bir_lowering=False, debug=False,
        dynamic_dma_scratch_size=32768,
    )

    xT = nc.dram_tensor("xT", [H, T], F32, kind="ExternalInput")
    WLT = nc.dram_tensor("WLT", [H, L], F32, kind="ExternalInput")
    Wc1T = nc.dram_tensor("Wc1T", [L, L], F32, kind="ExternalInput")
    Wc2T = nc.dram_tensor("Wc2T", [L, H], F32, kind="ExternalInput")
    Ws1T = nc.dram_tensor("Ws1T", [H, H], F32, kind="ExternalInput")
    Ws2T = nc.dram_tensor("Ws2T", [H, H], F32, kind="ExternalInput")
    gamma = nc.dram_tensor("gamma", [H], F32, kind="ExternalInput")
    beta = nc.dram_tensor("beta", [H], F32, kind="ExternalInput")
    s_largeT = nc.dram_tensor("s_largeT", [HO, LO], F32, kind="ExternalInput")
    s_s1T = nc.dram_tensor("s_s1T", [HO, HO], F32, kind="ExternalInput")
    s_s2T = nc.dram_tensor("s_s2T", [HO, HO], F32, kind="ExternalInput")
    out = nc.dram_tensor("out", [T, H], F32, kind="ExternalOutput")

    def tslab(dram_t, rows_off, nrows_k, col_off, width):
        # [128, nrows_k, width] view: partition p <- row k*128+p (+rows_off)
        return dram_t[:].rearrange("(k p) c -> p k c", p=P)[
            :, rows_off // P:rows_off // P + nrows_k, col_off:col_off + width
        ]

    with tile.TileContext(nc) as tc:
        with (
            tc.tile_pool(name="dram", bufs=1, space="DRAM") as dram,
            tc.tile_pool(name="consts", bufs=1) as consts,
            tc.tile_pool(name="wpool", bufs=2) as wpool,
            tc.tile_pool(name="apool", bufs=1) as apool,
            tc.tile_pool(name="lnp", bufs=4) as lnp,
            tc.tile_pool(name="psum", bufs=6, space="PSUM") as psum,
        ):
            # ---- DRAM scratch (dequanted / cast weight stripes) ----
            WTl = dram.tile([8, P, HO, 512], BF16)
            WTc1b = dram.tile([16, P, NBF, 256], BF16)
            WTc18 = dram.tile([16, P, NF8, 256], FP8)
            WTc2 = dram.tile([4, P, LO, 256], BF16)
            WTs = dram.tile([4, P, HO, 256], BF16)

            # ---- constants ----
            scA = consts.tile([P, HO, LO], F32)      # s_large^T  [k_h, l_blk]
            nc.sync.dma_start(
                scA[:], s_largeT[:].rearrange("a b -> (a b)")[None, :]
                .to_broadcast([P, HO * LO]).rearrange("p (a b) -> p a b", b=LO)
            )
            sc1 = consts.tile([P, HO, HO], F32)
            nc.sync.dma_start(
                sc1[:], s_s1T[:].rearrange("a b -> (a b)")[None, :]
                .to_broadcast([P, HO * HO]).rearrange("p (a b) -> p a b", b=HO)
            )
            sc2 = consts.tile([P, HO, HO], F32)
            nc.sync.dma_start(
                sc2[:], s_s2T[:].rearrange("a b -> (a b)")[None, :]
                .to_broadcast([P, HO * HO]).rearrange("p (a b) -> p a b", b=HO)
            )
            gamma_b = consts.tile([P, H], F32)
            nc.sync.dma_start(gamma_b[:], gamma[:][None, :].to_broadcast([P, H]))
            beta_b = consts.tile([P, H], F32)
            nc.sync.dma_start(beta_b[:], beta[:][None, :].to_broadcast([P, H]))
            eps_t = consts.tile([P, 1], F32)
            nc.vector.memset(eps_t[:], 1e-5)

            for tt in range(NTT):
                t0 = tt * TT
                first = tt == 0
                # x^T tile [h_in-part, h_blk, t] bf16, SWDGE cast-load
                xt = apool.tile([P, HO, TT], BF16, tag="xt", bufs=2, name="xt")
                nc.gpsimd.dma_start(xt[:], tslab(xT, 0, HO, t0, TT))

                # ---- Phase A: y1T[l, t] = relu(sum_h WLT[h, l] * xT[h, t]) ----
                y1b = apool.tile([P, NBF, TT], BF16, tag="y1b", bufs=1, name="y1b")
                y18 = apool.tile([P, NF8, TT], FP8, tag="y18", bufs=1, name="y18")
                with nc.named_scope(f"A{tt}"):
                    for j in range(8):          # l-stripes of 512
                        wl = wpool.tile([P, HO, 512], BF16, tag="wl", name="wl")
                        if first:
                            for hh in range(2):     # half-stripes of 256
                                t0a = wpool.tile([P, HO, 256], F32, tag="t0",
                                                 bufs=2, name="t0a")
                                nc.sync.dma_start(
                                    t0a[:], tslab(WLT, 0, HO, j * 512 + hh * 256, 256))
                                nc.vector.tensor_tensor(
                                    wl[:, :, hh * 256:(hh + 1) * 256]
                                    .rearrange("p k (b c) -> p k b c", c=P),
                                    t0a[:].rearrange("p k (b c) -> p k b c", c=P),
                                    scA[:, :, j * 4 + hh * 2:j * 4 + hh * 2 + 2]
                                    [:, :, :, None].to_broadcast([P, HO, 2, P]),
                                    MUL,
                                )
                            nc.sync.dma_start(WTl[j], wl[:])
                        else:
                            nc.sync.dma_start(wl[:], WTl[j])
                        for m in range(4):      # l-chunks of 128
                            jg = j * 4 + m      # B-contraction k-block id
                            ps = psum.tile([P, TT], F32, tag="ps", name="psA")
                            for k in range(HO):
                                nc.tensor.matmul(
                                    ps[:], wl[:, k, m * P:(m + 1) * P], xt[:, k, :],
                                    start=(k == 0), stop=(k == HO - 1),
                                )
                            if jg < NF8:
                                nc.scalar.activation(y18[:, jg, :], ps[:], Relu,
                                                     scale=SY)
                            else:
                                nc.scalar.activation(y1b[:, jg - NF8, :], ps[:], Relu)

                # ---- Phase B: y2T[l2, t] = relu(sum_l1 Wc1T[l1, l2] * y1T[l1, t]) ----
                y2T = apool.tile([P, LO, TT], BF16, tag="y2", bufs=1, name="y2T")
                with nc.named_scope(f"B{tt}"):
                    for j2 in range(16):        # l2-stripes of 256
                        wcb = wpool.tile([P, NBF, 256], BF16, tag="wcb", name="wcb")
                        wc8 = wpool.tile([P, NF8, 256], FP8, tag="wc8", bufs=1, name="wc8")
                        if first:
                            nc.gpsimd.dma_start(
                                wcb[:], tslab(Wc1T, NF8 * P, NBF, j2 * 256, 256))
                            nc.gpsimd.dma_start(WTc1b[j2], wcb[:])
                            for hh in range(NF8 // 8):
                                t0b = wpool.tile([P, 8, 256], F32, tag="t0",
                                                 bufs=2, name="t0b")
                                nc.scalar.dma_start(
                                    t0b[:], tslab(Wc1T, hh * 8 * P, 8, j2 * 256, 256))
                                nc.scalar.activation(
                                    wc8[:, hh * 8:(hh + 1) * 8, :], t0b[:], Copy,
                                    scale=SW)
                            nc.scalar.dma_start(WTc18[j2], wc8[:])
                        else:
                            nc.gpsimd.dma_start(wcb[:], WTc1b[j2])
                            nc.scalar.dma_start(wc8[:], WTc18[j2])
                        for m in range(2):      # l2-chunks of 128
                            ps = psum.tile([P, TT], F32, tag="ps", name="psB")
                            for kp in range(0, NF8, 2):
                                nc.tensor.matmul(
                                    ps[:], wc8[:, kp:kp + 2, m * P:(m + 1) * P],
                                    y18[:, kp:kp + 2, :],
                                    start=(kp == 0), stop=False, perf_mode=DR,
                                )
                            for k in range(NBF):
                                nc.tensor.matmul(
                                    ps[:], wcb[:, k, m * P:(m + 1) * P], y1b[:, k, :],
                                    start=False, stop=(k == NBF - 1),
                                )
                            nc.scalar.activation(y2T[:, j2 * 2 + m, :], ps[:], Relu)

                # ---- Phase C: pre[t, h] = sum_l2 y2[l2, t]*Wc2T[l2, h] + skip ----
                preln = apool.tile([P, TT // P, H], F32, tag="ln", bufs=1, name="preln")
                with nc.named_scope(f"C{tt}"):
                    for jh in range(4):         # h-blocks of 256
                        wc2s = wpool.tile([P, LO, 256], BF16, tag="wc2s", name="wc2s")
                        ws = wpool.tile([P, HO, 256], BF16, tag="ws", bufs=1, name="ws")
                        if first:
                            nc.gpsimd.dma_start(
                                wc2s[:], tslab(Wc2T, 0, LO, jh * 256, 256))
                            nc.gpsimd.dma_start(WTc2[jh], wc2s[:])
                            w1 = wpool.tile([P, HO, 256], F32, tag="t0", bufs=2,
                                            name="w1")
                            nc.scalar.dma_start(
                                w1[:], tslab(Ws1T, 0, HO, jh * 256, 256))
                            w2 = wpool.tile([P, HO, 256], F32, tag="t0", bufs=2,
                                            name="w2")
                            nc.scalar.dma_start(
                                w2[:], tslab(Ws2T, 0, HO, jh * 256, 256))
                            for w, sc in ((w1, sc1), (w2, sc2)):
                                nc.vector.tensor_tensor(
                                    w[:].rearrange("p k (b c) -> p k b c", c=P),
                                    w[:].rearrange("p k (b c) -> p k b c", c=P),
                                    sc[:, :, jh * 2:jh * 2 + 2]
                                    [:, :, :, None].to_broadcast([P, HO, 2, P]),
                                    MUL,
                                )
                            nc.vector.tensor_tensor(ws[:], w1[:], w2[:], ADD)
                            nc.scalar.dma_start(WTs[jh], ws[:])
                        else:
                            nc.gpsimd.dma_start(wc2s[:], WTc2[jh])
                            nc.scalar.dma_start(ws[:], WTs[jh])
                        for tn in range(TT // P):   # token chunks of 128
                            ps = psum.tile([P, 256], F32, tag="ps", name="psC")
                            for k in range(LO):
                                nc.tensor.matmul(
                                    ps[:], y2T[:, k, tn * P:(tn + 1) * P],
                                    wc2s[:, k, :],
                                    start=(k == 0), stop=False,
                                )
                            for k in range(HO):
                                nc.tensor.matmul(
                                    ps[:], xt[:, k, tn * P:(tn + 1) * P], ws[:, k, :],
                                    start=False, stop=(k == HO - 1),
                                )
                            nc.vector.tensor_copy(
                                preln[:, tn, jh * 256:(jh + 1) * 256], ps[:]
                            )

                # ---- Phase D: layernorm over h, * gamma + beta ----
                with nc.named_scope(f"D{tt}"):
                    for tn in range(TT // P):
                        pslice = preln[:, tn, :]
                        st = lnp.tile([P, 2, 6], F32, name="st")
                        for g in range(2):
                            nc.vector.bn_stats(st[:, g, :], pslice[:, g * 512:(g + 1) * 512])
                        mv = lnp.tile([P, 2], F32, name="mv")
                        nc.vector.bn_aggr(mv[:], st[:])
                        std = lnp.tile([P, 1], F32, name="std")
                        nc.scalar.activation(std[:], mv[:, 1:2], Sqrt, bias=eps_t[:])
                        rstd = lnp.tile([P, 1], F32, name="rstd")
                        nc.vector.reciprocal(rstd[:], std[:])
                        nc.vector.tensor_scalar(
                            pslice, pslice, scalar1=mv[:, 0:1], scalar2=rstd[:],
                            op0=SUB, op1=MUL,
                        )
                        nc.vector.tensor_tensor(pslice, pslice, gamma_b[:], MUL)
                        nc.vector.tensor_tensor(pslice, pslice, beta_b[:], ADD)
                        nc.sync.dma_start(out[t0 + tn * P:t0 + (tn + 1) * P, :], pslice)

    nc.finalize()
    return nc


def _get_nc():
    global _CACHED_NC
    if _CACHED_NC is None:
        _CACHED_NC = _build_nc()
    return _CACHED_NC


def prepare_in_maps(inputs):
    f32 = np.float32
    x = np.asarray(inputs["x"], dtype=f32)                 # [8, T, H]
    xT_all = np.ascontiguousarray(x.transpose(0, 2, 1))    # [8, H, T]
    shared = {
        "WLT": np.ascontiguousarray(np.asarray(inputs["W_large"], f32).T),
        "Wc1T": np.ascontiguousarray(np.asarray(inputs["W_c1"], f32).T),
        "Wc2T": np.ascontiguousarray(np.asarray(inputs["W_c2"], f32).T),
        "Ws1T": np.ascontiguousarray(np.asarray(inputs["W_s1"], f32).T),
        "Ws2T": np.ascontiguousarray(np.asarray(inputs["W_s2"], f32).T),
        "gamma": np.ascontiguousarray(np.asarray(inputs["gamma"], f32)),
        "beta": np.ascontiguousarray(np.asarray(inputs["beta"], f32)),
        "s_largeT": np.ascontiguousarray(np.asarray(inputs["s_large"], f32).T),
        "s_s1T": np.ascontiguousarray(np.asarray(inputs["s_s1"], f32).T),
        "s_s2T": np.ascontiguousarray(np.asarray(inputs["s_s2"], f32).T),
    }
    return [{"xT": np.ascontiguousarray(xT_all[c]), **shared}
            for c in range(NCORES)]


def kernel(**inputs) -> np.ndarray:
    from concourse.bass_utils import run_bass_kernel_spmd

    nc = _get_nc()
    in_maps = prepare_in_maps(inputs)
    res = run_bass_kernel_spmd(nc, in_maps, core_ids=list(range(NCORES)))
    return np.stack([res.results[c]["out"] for c in range(NCORES)], axis=0)
